# revision 15
# baseline (speedup 1.0000x reference)
"""Trainium2 Bass kernel for CorpusSupportSets RBF tangent-field.

Math per sample row i (dim 768), with one-hot mask selecting dipole k:
    k    = argmax(mask[i])            (exact: dot with iota row)
    s0,s1 = SUPPORT_SETS[k] halves;  a_j = ALPHAS[k,j];  g_j = exp(LOGGAMMA[k,j])
    zz = |z|^2, ss_j = |s_j|^2, t_j = z.s_j, n_j = zz - 2 t_j + ss_j
    m_j = a_j * g_j * exp(-g_j n_j)
    beta = (m0+m1)(zz-1) - m0 t0 - m1 t1
    p    = beta z + m0 s0 + m1 s1     (= -proj/2 of the reference, sign-safe)
    out  = p / |p|

Sharding: data-parallel over batch across 8 cores (2048 rows each).
The one-hot matmuls of the reference are replaced by an exact index
computation + indirect-DMA row gather from a host-concatenated table
[SUPPORT_SETS | ALPHAS | LOGGAMMA] of shape [1000, 1540].
"""
import sys

for _p in ("/opt/trn_rl_repo",):
    if _p not in sys.path:
        sys.path.insert(0, _p)

import numpy as np

import concourse.bass as bass
import concourse.tile as tile
from concourse import mybir
from concourse.bass import IndirectOffsetOnAxis
from concourse.bass_utils import run_bass_kernel_spmd
from concourse.vector_clock import ScopedClock

# ---------------------------------------------------------------------------
# Workaround: this walrus build only accepts ONE semaphore wait per
# instruction; the TileContext exit drain accumulates one wait per live
# semaphore lane.  Split overflow waits onto trailing sync-engine NOPs.
_MAX_WAITS = 1


def _split_waits(nc, inst):
    si = inst.sync_info
    if si is None:
        return
    waits = list(si.on_wait)
    if len(waits) <= _MAX_WAITS:
        return
    inst.sync_info = mybir.SyncInfo(
        on_wait=waits[:_MAX_WAITS], on_update=list(si.on_update)
    )
    for i in range(_MAX_WAITS, len(waits), _MAX_WAITS):
        nop = nc.sync.nop(nofuse=True, hint="drain_wait_overflow")
        nop.ins.sync_info = mybir.SyncInfo(
            on_wait=waits[i : i + _MAX_WAITS], on_update=[]
        )


def _patched_drain_and_barrier(self, tick_clock, wait_clock):
    drain_inst = self.nc.sync.drain()
    wait_clock.add_sem_waits(
        drain_inst.ins, ScopedClock({None: tick_clock.global_clock})
    )
    _split_waits(self.nc, drain_inst.ins)
    self.nc.all_engine_barrier()
    assert self.sems is not None
    popped = self.nc._tile_sem_poison_stack.pop()
    assert popped is self._sem_poison
    self.nc.clear_and_free_semaphores(list(self.sems.allocated().values()))
    self.nc.all_engine_barrier()


_orig_commit = tile.TileContext._commit_instruction


def _patched_commit(self, inst, lazy_reg_writes=True):
    si = getattr(inst, "sync_info", None)
    if (
        si is not None
        and si.on_wait
        and len(si.on_wait) > _MAX_WAITS
        and inst.engine != mybir.EngineType.Unassigned
    ):
        waits = list(si.on_wait)
        inst.sync_info = mybir.SyncInfo(
            on_wait=waits[:_MAX_WAITS], on_update=list(si.on_update)
        )
        for _i, _w in enumerate(waits[_MAX_WAITS:]):
            nop = mybir.InstNoOp(
                name=f"{inst.name}_w{_i}",
                engine=inst.engine,
                sync_info=mybir.SyncInfo(on_wait=[_w], on_update=[]),
                bass_nofuse=True,
            )
            self._add_instruction(nop)
    return _orig_commit(self, inst, lazy_reg_writes)


tile.TileContext._drain_and_barrier = _patched_drain_and_barrier
tile.TileContext._commit_instruction = _patched_commit

# ---------------------------------------------------------------------------
BS, K, DIM = 16384, 1000, 768
NCORES = 8
ROWS = BS // NCORES  # 2048 rows per core
P = 128
NT = ROWS // P  # 16 tiles of 128 rows
GRP = 4  # tiles per group
NG = NT // GRP  # 4 groups
TBL_W = 2 * DIM + 4  # 1540: [s0 | s1 | a0 a1 lg0 lg1]
F32 = mybir.dt.float32
U32 = mybir.dt.uint32


def build_nc(rows=ROWS):
    NT = rows // P
    NG = NT // GRP
    OP = mybir.AluOpType
    AT = mybir.ActivationFunctionType
    BF16 = mybir.dt.bfloat16
    nc = bass.Bass()
    zin = nc.dram_tensor("zin", [rows, DIM], F32, kind="ExternalInput")
    mk = nc.dram_tensor("mk", [rows, K], BF16, kind="ExternalInput")
    tbl = nc.dram_tensor("tbl", [K, TBL_W], F32, kind="ExternalInput")
    out = nc.dram_tensor("out", [rows, DIM], F32, kind="ExternalOutput")

    with tile.TileContext(nc) as tc:
        with (
            tc.tile_pool(name="zp", bufs=3) as zp,
            tc.tile_pool(name="mkp", bufs=2) as mkp,
            tc.tile_pool(name="selp", bufs=3) as selp,
            tc.tile_pool(name="outp", bufs=2) as outp,
            tc.tile_pool(name="scrD", bufs=4, space="PSUM") as scrDp,
            tc.tile_pool(name="wp", bufs=4) as wp,
            tc.tile_pool(name="tiny", bufs=40) as tinyp,
            tc.tile_pool(name="singles", bufs=1) as singles,
        ):
            ss0a = singles.tile([P, NT], F32)
            ss1a = singles.tile([P, NT], F32)
            q0a = singles.tile([P, NT], F32)
            q1a = singles.tile([P, NT], F32)
            pna = singles.tile([P, NT], F32)
            sqa = singles.tile([P, NT], F32)
            ra = singles.tile([P, NT], F32)
            sidea = singles.tile([P, NT, 4], F32)
            mia = singles.tile([P, NT, 8], U32)

            def phase1(g):
                r0, r1 = g * GRP * P, (g + 1) * GRP * P
                c0, c1 = g * GRP, (g + 1) * GRP
                z_g = zp.tile([P, GRP, DIM], F32, name="z_g", tag="z")
                nc.sync.dma_start(
                    out=z_g[:], in_=zin[r0:r1].rearrange("(n p) c -> p n c", p=P)
                )
                mk_g = mkp.tile([P, GRP, K], mybir.dt.bfloat16, name="mk_g", tag="mk")
                nc.sync.dma_start(
                    out=mk_g[:], in_=mk[r0:r1].rearrange("(n p) c -> p n c", p=P)
                )
                # argmax of one-hot mask, gather table rows into one tile
                sel4 = selp.tile([P, GRP, TBL_W], F32, name="sel4", tag="sel")
                for n in range(GRP):
                    j = c0 + n
                    mx = tinyp.tile([P, 8], mybir.dt.bfloat16, name="mx", tag="mx")
                    nc.vector.max(out=mx[:], in_=mk_g[:, n, :])
                    nc.vector.max_index(
                        out=mia[:, j, :], in_max=mx[:], in_values=mk_g[:, n, :]
                    )
                    nc.gpsimd.indirect_dma_start(
                        out=sel4[:, n, :],
                        out_offset=None,
                        in_=tbl[:],
                        in_offset=IndirectOffsetOnAxis(ap=mia[:, j, 0:1], axis=0),
                    )
                # batched w = z + s for both poles (single big DVE ops)
                w0 = wp.tile([P, GRP, DIM], F32, name="w0", tag="w")
                nc.vector.tensor_tensor(
                    out=w0[:], in0=z_g[:], in1=sel4[:, :, :DIM], op=OP.add
                )
                w1 = wp.tile([P, GRP, DIM], F32, name="w1", tag="w")
                nc.vector.tensor_tensor(
                    out=w1[:], in0=z_g[:], in1=sel4[:, :, DIM : 2 * DIM], op=OP.add
                )
                # per-row reductions on ACT (accumulate along free axis)
                for n in range(GRP):
                    j = c0 + n
                    nc.scalar.activation(
                        out=scrDp.tile([P, DIM], F32, name="scrd", tag="scrD")[:],
                        in_=sel4[:, n, :DIM], func=AT.Square,
                        accum_out=ss0a[:, j : j + 1],
                    )
                    nc.scalar.activation(
                        out=scrDp.tile([P, DIM], F32, name="scrd", tag="scrD")[:],
                        in_=sel4[:, n, DIM : 2 * DIM], func=AT.Square,
                        accum_out=ss1a[:, j : j + 1],
                    )
                    nc.scalar.activation(
                        out=scrDp.tile([P, DIM], F32, name="scrd", tag="scrD")[:],
                        in_=w0[:, n, :], func=AT.Square, accum_out=q0a[:, j : j + 1],
                    )
                    nc.scalar.activation(
                        out=scrDp.tile([P, DIM], F32, name="scrd", tag="scrD")[:],
                        in_=w1[:, n, :], func=AT.Square, accum_out=q1a[:, j : j + 1],
                    )
                nc.gpsimd.tensor_copy(
                    out=sidea[:, c0:c1, :], in_=sel4[:, :, 2 * DIM :]
                )

                # per-group small math on [P, GRP] columns
                def _m(qv, ssv, av, lgv, eng):
                    gt = tinyp.tile([P, GRP], F32, name="gt", tag="tiny")
                    nc.scalar.activation(out=gt[:], in_=lgv, func=AT.Exp)
                    d = tinyp.tile([P, GRP], F32, name="d", tag="tiny")
                    eng.tensor_scalar(
                        out=d[:], in0=ssv, scalar1=1.0, scalar2=None, op0=OP.add
                    )
                    t2 = tinyp.tile([P, GRP], F32, name="t2", tag="tiny")
                    eng.tensor_tensor(out=t2[:], in0=qv, in1=d[:], op=OP.subtract)
                    nn = tinyp.tile([P, GRP], F32, name="nn", tag="tiny")
                    eng.tensor_scalar(
                        out=nn[:], in0=d[:], scalar1=2.0, scalar2=None, op0=OP.mult
                    )
                    eng.tensor_tensor(out=nn[:], in0=nn[:], in1=qv, op=OP.subtract)
                    eng.tensor_tensor(out=nn[:], in0=nn[:], in1=gt[:], op=OP.mult)
                    e = tinyp.tile([P, GRP], F32, name="e", tag="tiny")
                    nc.scalar.activation(out=e[:], in_=nn[:], func=AT.Exp, scale=-1.0)
                    m = tinyp.tile([P, GRP], F32, name="m", tag="tiny")
                    eng.tensor_tensor(out=m[:], in0=e[:], in1=gt[:], op=OP.mult)
                    eng.tensor_tensor(out=m[:], in0=m[:], in1=av, op=OP.mult)
                    return m, t2

                m0, t20 = _m(
                    q0a[:, c0:c1], ss0a[:, c0:c1],
                    sidea[:, c0:c1, 0], sidea[:, c0:c1, 2], nc.vector,
                )
                m1, t21 = _m(
                    q1a[:, c0:c1], ss1a[:, c0:c1],
                    sidea[:, c0:c1, 1], sidea[:, c0:c1, 3], nc.gpsimd,
                )
                # beta = -(m0*t20 + m1*t21)/2   (zz == 1)
                h0 = tinyp.tile([P, GRP], F32, name="h0", tag="tiny")
                nc.vector.tensor_tensor(out=h0[:], in0=m0[:], in1=t20[:], op=OP.mult)
                h1 = tinyp.tile([P, GRP], F32, name="h1", tag="tiny")
                nc.gpsimd.tensor_tensor(out=h1[:], in0=m1[:], in1=t21[:], op=OP.mult)
                bB = tinyp.tile([P, GRP], F32, name="bB", tag="tiny")
                nc.vector.tensor_tensor(out=bB[:], in0=h0[:], in1=h1[:], op=OP.add)
                nc.vector.tensor_scalar(
                    out=bB[:], in0=bB[:], scalar1=-0.5, scalar2=None, op0=OP.mult
                )
                return dict(g=g, z_g=z_g, sel4=sel4, m0=m0, m1=m1, bB=bB)

            def phase2(st):
                g = st["g"]
                r0, r1 = g * GRP * P, (g + 1) * GRP * P
                c0, c1 = g * GRP, (g + 1) * GRP
                z_g, sel4, m0, m1, bB = (
                    st["z_g"], st["sel4"], st["m0"], st["m1"], st["bB"]
                )
                pg = outp.tile([P, GRP, DIM], F32, name="pg", tag="pg")
                for n in range(GRP):
                    j = c0 + n
                    p_n = pg[:, n, :]
                    nc.vector.tensor_scalar(
                        out=p_n, in0=z_g[:, n, :], scalar1=bB[:, n : n + 1],
                        scalar2=None, op0=OP.mult,
                    )
                    nc.vector.scalar_tensor_tensor(
                        out=p_n, in0=sel4[:, n, :DIM], scalar=m0[:, n : n + 1],
                        in1=p_n, op0=OP.mult, op1=OP.add,
                    )
                    nc.vector.scalar_tensor_tensor(
                        out=p_n, in0=sel4[:, n, DIM : 2 * DIM],
                        scalar=m1[:, n : n + 1],
                        in1=p_n, op0=OP.mult, op1=OP.add,
                    )
                    nc.scalar.activation(
                        out=scrDp.tile([P, DIM], F32, name="scrd", tag="scrD")[:],
                        in_=p_n, func=AT.Square, accum_out=pna[:, j : j + 1],
                    )
                nc.scalar.activation(
                    out=sqa[:, c0:c1], in_=pna[:, c0:c1], func=AT.Sqrt
                )
                nc.vector.reciprocal(out=ra[:, c0:c1], in_=sqa[:, c0:c1])
                for n in range(GRP):
                    j = c0 + n
                    nc.vector.tensor_scalar(
                        out=pg[:, n, :], in0=pg[:, n, :], scalar1=ra[:, j : j + 1],
                        scalar2=None, op0=OP.mult,
                    )
                nc.sync.dma_start(
                    out=out[r0:r1].rearrange("(n p) c -> p n c", p=P), in_=pg[:]
                )

            pending = []
            for g in range(NG):
                pending.append(phase1(g))
                if len(pending) > 2:
                    phase2(pending.pop(0))
            for st in pending:
                phase2(st)
    return nc


_NC_CACHE = None


def _get_nc():
    global _NC_CACHE
    if _NC_CACHE is None:
        _NC_CACHE = build_nc()
    return _NC_CACHE


def build_in_maps(inputs):
    import ml_dtypes

    z = np.ascontiguousarray(inputs["z"], dtype=np.float32)
    mask = np.asarray(inputs["support_sets_mask"], dtype=np.float32)
    mk = mask.astype(ml_dtypes.bfloat16)
    tbl = np.ascontiguousarray(
        np.concatenate(
            [
                np.asarray(inputs["SUPPORT_SETS"], dtype=np.float32),
                np.asarray(inputs["ALPHAS"], dtype=np.float32),
                np.asarray(inputs["LOGGAMMA"], dtype=np.float32),
            ],
            axis=1,
        )
    )
    return [
        {
            "zin": np.ascontiguousarray(z[c * ROWS : (c + 1) * ROWS]),
            "mk": np.ascontiguousarray(mk[c * ROWS : (c + 1) * ROWS]),
            "tbl": tbl,
        }
        for c in range(NCORES)
    ]


def kernel(support_sets_mask, z, SUPPORT_SETS, ALPHAS, LOGGAMMA):
    in_maps = build_in_maps(
        dict(
            support_sets_mask=support_sets_mask, z=z,
            SUPPORT_SETS=SUPPORT_SETS, ALPHAS=ALPHAS, LOGGAMMA=LOGGAMMA,
        )
    )
    nc = _get_nc()
    res = run_bass_kernel_spmd(nc, in_maps, list(range(NCORES)))
    return np.concatenate([res.results[c]["out"] for c in range(NCORES)], axis=0)
